# revision 27
# baseline (speedup 1.0000x reference)
"""Bone_Direction_GCN fused kernel for 8 Trainium2 NeuronCores.

Data-parallel over the batch dim: each core processes 2048 of 16384 batches.
x is shipped to the device as bf16 channel-major [CIN, rows] so both the input
and output DMAs move large contiguous per-partition chunks (~7.6KB packets).
Graph mixing (GCN conv + dense-adj einsum) is expressed as block-diagonal
"mixing transpose" matmuls over groups of 7 batches (7*17 = 119 rows), which
also return the result to channel-major layout for the residual add and the
bf16 channel-major output store.
"""

import sys

sys.path.insert(0, "/opt/trn_rl_repo")

import numpy as np
import ml_dtypes

B, J, E = 16384, 17, 32
CIN, COUT = 128, 128
MID = COUT // 2
PROP = 0.5
SLOPE = 0.01

N_CORES = 8
BC = B // N_CORES          # batches per core (2048)
ROWS = BC * J              # rows per core (34816)
G = 7                      # batches per group
R = G * J                  # rows per group (119)
NG = BC // G               # full groups per core (292)
GT = BC - NG * G           # tail batches (4)
RT = GT * J                # tail rows (68)
SGS = 4                    # groups per supergroup
RSG = SGS * R              # cols per supergroup (476)
NSG = NG // SGS            # supergroups per core (73)
# tiles by supergroup count: big in steady state, descending at the end so
# the final output DMAs are small (short drain)
TILE_SGS = [8, 8, 8, 8, 8, 8, 8, 8, 4, 2, 2, 1]
assert sum(TILE_SGS) == NSG
CB_COLS = 1206             # packed bf16 const tensor columns

assert sum(TILE_SGS) * RSG + RT == ROWS

_CACHE = {}


def _gcn_matrix(edge_index: np.ndarray, edge_weight: np.ndarray) -> np.ndarray:
    """Dense normalized GCN operator M with out[i] = sum_j M[i, j] * x[j]."""
    row = edge_index[0].astype(np.int64)
    col = edge_index[1].astype(np.int64)
    loop = np.arange(J, dtype=np.int64)
    row_f = np.concatenate([row, loop])
    col_f = np.concatenate([col, loop])
    w_f = np.concatenate([edge_weight.astype(np.float32), np.ones(J, np.float32)])
    deg = np.zeros(J, np.float32)
    np.add.at(deg, col_f, w_f)
    safe = np.where(deg > 0, deg, 1.0).astype(np.float32)
    dis = np.where(deg > 0, 1.0 / np.sqrt(safe), 0.0).astype(np.float32)
    norm = dis[row_f] * w_f * dis[col_f]
    M = np.zeros((J, J), np.float32)
    np.add.at(M, (col_f, row_f), norm)
    return M


def _block_diag(block: np.ndarray, n: int) -> np.ndarray:
    j = block.shape[0]
    out = np.zeros((n * j, n * j), block.dtype)
    for g in range(n):
        out[g * j:(g + 1) * j, g * j:(g + 1) * j] = block
    return out


def _mix_consts(M: np.ndarray, adj: np.ndarray, g: int):
    """bdM [g*17, g*17] = blockdiag(M.T); m2e [g*17+2, g*17] = mix2 + bias rows.

    psF[c, r'] += sum_r y1u[r, c] * bdM[r, r']   (GCN mix, row r -> row r')
    psF[c, r'] += sum_k y2e[k, c] * m2e[k, r']   (adj mix + b1/b4 bias rows)
    """
    r = g * J
    bdM = _block_diag(M.T, g)
    mix2 = _block_diag(PROP * adj, g)
    ones_row = np.ones((1, r), np.float32)
    s_row = np.tile(PROP * adj.sum(axis=0), g)[None, :]
    m2e = np.concatenate([mix2, ones_row, s_row], axis=0)
    return bdM, m2e


def _build_bass(leaky_mode: str = "lrelu", **_ignored):
    import concourse.bacc as bacc
    import concourse.mybir as mybir
    import concourse.tile as tile
    from contextlib import ExitStack

    f32 = mybir.dt.float32
    bf16 = mybir.dt.bfloat16

    nc = bacc.Bacc("TRN2", target_bir_lowering=False, debug=False)

    x_d = nc.dram_tensor("x", [CIN, ROWS], bf16, kind="ExternalInput").ap()
    # all bf16 consts packed into one tensor = one DMA at startup
    cb_d = nc.dram_tensor("cb", [CIN, CB_COLS], bf16, kind="ExternalInput").ap()
    b2ab2_d = nc.dram_tensor("b2ab2", [MID, 2], f32, kind="ExternalInput").ap()
    o_d = nc.dram_tensor("out", [CIN, ROWS], bf16, kind="ExternalOutput").ap()

    with ExitStack() as ctx:
        tc = ctx.enter_context(tile.TileContext(nc))

        const = ctx.enter_context(tc.tile_pool(name="const", bufs=1))
        cb_sb = const.tile([CIN, CB_COLS], bf16, name="cb_sb")
        nc.scalar.dma_start(out=cb_sb[:], in_=cb_d)
        b2ab2_sb = const.tile([MID, 2], f32, name="b2ab2_sb")
        nc.scalar.dma_start(out=b2ab2_sb[:], in_=b2ab2_d)
        w1_sb = cb_sb[:, 0:128]
        w2t_sb = cb_sb[:, 128:192]
        w4t_sb = cb_sb[0:MID, 192:320]
        bdM_sb = cb_sb[0:R, 320:439]
        m2e_sb = cb_sb[0:R + 2, 439:558]
        bdM4_sb = cb_sb[0:RT, 558:626]
        m2e4_sb = cb_sb[0:RT + 2, 626:694]
        b2_sb = b2ab2_sb[:, 0:1]
        ab2_sb = b2ab2_sb[:, 1:2]

        def leaky(hbf, psH):
            if leaky_mode == "lrelu":
                nc.scalar.activation(
                    hbf[:], psH[:],
                    func=mybir.ActivationFunctionType.Lrelu,
                    bias=b2_sb[:], scale=1.0, alpha=SLOPE,
                )
            else:
                a = lk_pool.tile(list(psH.shape), bf16, tag="lk_a")
                nc.scalar.activation(
                    a[:], psH[:],
                    func=mybir.ActivationFunctionType.Identity,
                    bias=ab2_sb[:], scale=SLOPE,
                )
                nc.vector.scalar_tensor_tensor(
                    hbf[:], psH[:], b2_sb[:], a[:],
                    op0=mybir.AluOpType.add, op1=mybir.AluOpType.max,
                )

        # y2e tiles: rows 0:R hold y2 (d before bias/mix); rows R:R+2 hold b1/b4
        y2e_pool = ctx.enter_context(tc.tile_pool(name="y2e", bufs=3))
        y2e_tiles = []
        for i in range(3):
            t = y2e_pool.tile([R + 2, SGS * COUT], bf16, tag=f"y2e{i}")
            nc.scalar.dma_start(out=t[R:R + 2, :], in_=cb_d[0:2, 694:694 + 512])
            y2e_tiles.append(t)
        y2et_pool = ctx.enter_context(tc.tile_pool(name="y2et", bufs=1))
        y2et = y2et_pool.tile([RT + 2, COUT], bf16)
        nc.scalar.dma_start(out=y2et[RT:RT + 2, :], in_=cb_d[0:2, 694:694 + COUT])

        xin_pool = ctx.enter_context(tc.tile_pool(name="xin", bufs=3))
        fout_pool = ctx.enter_context(tc.tile_pool(name="fout", bufs=3))
        h_pool = ctx.enter_context(tc.tile_pool(name="h", bufs=3))
        y1u_pool = ctx.enter_context(tc.tile_pool(name="y1u", bufs=3))
        lk_pool = ctx.enter_context(tc.tile_pool(name="lk", bufs=2))

        psH_pool = ctx.enter_context(tc.tile_pool(name="psH", bufs=2, space="PSUM"))
        psA_pool = ctx.enter_context(tc.tile_pool(name="psA", bufs=2, space="PSUM"))
        psB_pool = ctx.enter_context(tc.tile_pool(name="psB", bufs=2, space="PSUM"))
        psF_pool = ctx.enter_context(tc.tile_pool(name="psF", bufs=2, space="PSUM"))

        # tiles: (col_start, ncols, n_supergroups); the 68-col tail of x is a
        # separate mini tile processed first (fills the DMA warmup bubble)
        tiles = []
        c0 = 0
        for nsg in TILE_SGS:
            tiles.append((c0, nsg * RSG, nsg))
            c0 += nsg * RSG
        tiles.append((c0, RT, 0))  # tail mini tile
        # supergroup s -> (tile_idx, col offset within tile)
        sg_map = []
        for ti, (c0, ncols, nsg) in enumerate(tiles):
            for k in range(nsg):
                sg_map.append((ti, k * RSG))

        xin_tiles = [None] * len(tiles)
        fout_tiles = [None] * len(tiles)
        h_tiles = [None] * len(tiles)

        def open_tile(ti, bounds=None):
            c0, ncols, _ = tiles[ti]
            tag = f"c{ncols}"
            xt = xin_pool.tile([CIN, ncols], bf16, tag=f"x{tag}", name=f"xin_{tag}")
            # chunked loads so downstream compute starts on the first chunk
            if bounds is None:
                bounds = [0, (ncols + 1) // 2, ncols] if ncols > RSG else [0, ncols]
            for lo, hi in zip(bounds, bounds[1:]):
                nc.sync.dma_start(out=xt[:, lo:hi], in_=x_d[:, c0 + lo:c0 + hi])
            xin_tiles[ti] = xt
            fout_tiles[ti] = fout_pool.tile(
                [CIN, ncols], bf16, tag=f"f{tag}", name=f"fout_{tag}")
            h_tiles[ti] = h_pool.tile(
                [MID, ncols], bf16, tag=f"h{tag}", name=f"h_{tag}")

        def emit_w2(s):
            ti, off = sg_map[s]
            xt, ht = xin_tiles[ti], h_tiles[ti]
            psH = psH_pool.tile([MID, RSG], f32, tag="psH")
            nc.tensor.matmul(psH[:], lhsT=w2t_sb[:], rhs=xt[:, off:off + RSG],
                             start=True, stop=True)
            leaky(ht[:, off:off + RSG], psH)

        # stage 2 state: psF + supergroup id, lagged by one iteration
        pend = []
        adds_done = [0] * len(tiles)

        def emit_front(s):
            """G1/G2 matmuls + copies for supergroup s."""
            ti, off = sg_map[s]
            xt, ht = xin_tiles[ti], h_tiles[ti]
            psA = psA_pool.tile([R, SGS * COUT], f32, tag="psA")
            for i in range(SGS):
                nc.tensor.matmul(
                    psA[:, i * COUT:(i + 1) * COUT],
                    lhsT=xt[:, off + i * R:off + (i + 1) * R], rhs=w1_sb[:],
                    start=True, stop=True)
            y1u = y1u_pool.tile([R, SGS * COUT], bf16, tag="y1u")
            nc.vector.tensor_copy(y1u[:], psA[:])
            psB = psB_pool.tile([R, SGS * COUT], f32, tag="psB")
            for i in range(SGS):
                nc.tensor.matmul(
                    psB[:, i * COUT:(i + 1) * COUT],
                    lhsT=ht[:, off + i * R:off + (i + 1) * R], rhs=w4t_sb[:],
                    start=True, stop=True)
            y2e = y2e_tiles[s % 3]
            nc.scalar.copy(y2e[0:R, :], psB[:])
            pend.append((s, y1u, y2e))

        def emit_back():
            """Mixing transposes + residual add for the oldest pending SG."""
            s, y1u, y2e = pend.pop(0)
            ti, off = sg_map[s]
            xt, ft = xin_tiles[ti], fout_tiles[ti]
            psF = psF_pool.tile([COUT, RSG], f32, tag="psF")
            for i in range(SGS):
                nc.tensor.matmul(
                    psF[:, i * R:(i + 1) * R],
                    lhsT=y1u[:, i * COUT:(i + 1) * COUT], rhs=bdM_sb[:],
                    start=True, stop=False, skip_group_check=True)
                nc.tensor.matmul(
                    psF[:, i * R:(i + 1) * R],
                    lhsT=y2e[:, i * COUT:(i + 1) * COUT], rhs=m2e_sb[:],
                    start=False, stop=True, skip_group_check=True)
            nc.vector.tensor_add(ft[:, off:off + RSG], psF[:], xt[:, off:off + RSG])
            adds_done[ti] += 1
            nsg = tiles[ti][2]
            if nsg >= 2 and adds_done[ti] == nsg // 2:
                close_cols(ti, 0, (nsg // 2) * RSG)
            elif adds_done[ti] == nsg:
                lo = (nsg // 2) * RSG if nsg >= 2 else 0
                close_cols(ti, lo, tiles[ti][1])

        def close_cols(ti, lo, hi):
            c0 = tiles[ti][0]
            nc.scalar.dma_start(
                out=o_d[:, c0 + lo:c0 + hi], in_=fout_tiles[ti][:, lo:hi])

        def emit_tail():
            """Tail group: 4 batches / 68 rows in its own mini tile, emitted
            first so it runs during the DMA warmup bubble."""
            ti = len(tiles) - 1
            xt, ht, ft = xin_tiles[ti], h_tiles[ti], fout_tiles[ti]
            psHt = psH_pool.tile([MID, RSG], f32, tag="psH")
            psH = psHt[:, 0:RT]
            nc.tensor.matmul(psH, lhsT=w2t_sb[:], rhs=xt[:],
                             start=True, stop=True)
            leaky(ht[:], psH)
            psAt = psA_pool.tile([R, SGS * COUT], f32, tag="psA")
            psA = psAt[0:RT, 0:COUT]
            nc.tensor.matmul(psA, lhsT=xt[:], rhs=w1_sb[:],
                             start=True, stop=True)
            y1u = y1u_pool.tile([RT, COUT], bf16, tag="y1ut")
            nc.vector.tensor_copy(y1u[:], psA)
            psBt = psB_pool.tile([R, SGS * COUT], f32, tag="psB")
            psB = psBt[0:RT, 0:COUT]
            nc.tensor.matmul(psB, lhsT=ht[:], rhs=w4t_sb[:],
                             start=True, stop=True)
            nc.scalar.copy(y2et[0:RT, :], psB)
            psFt = psF_pool.tile([COUT, RSG], f32, tag="psF")
            psF = psFt[:, 0:RT]
            nc.tensor.matmul(psF, lhsT=y1u[:], rhs=bdM4_sb[:],
                             start=True, stop=False, skip_group_check=True)
            nc.tensor.matmul(psF, lhsT=y2et[:], rhs=m2e4_sb[:],
                             start=False, stop=True, skip_group_check=True)
            nc.vector.tensor_add(ft[:], psF, xt[:])
            close_cols(ti, 0, RT)

        open_tile(len(tiles) - 1)   # tail mini tile
        open_tile(0, bounds=[0, RSG, 2 * RSG, 4 * RSG, 6 * RSG, 8 * RSG])
        emit_tail()
        emit_w2(0)
        if NSG > 1:
            if sg_map[1][0] != 0 and xin_tiles[sg_map[1][0]] is None:
                open_tile(sg_map[1][0])
            emit_w2(1)
        for s in range(NSG):
            emit_front(s)
            if pend and pend[0][0] < s - 1:
                emit_back()
            if s + 2 < NSG:
                if xin_tiles[sg_map[s + 2][0]] is None:
                    open_tile(sg_map[s + 2][0])
                emit_w2(s + 2)
        while pend:
            emit_back()
        assert adds_done[:-1] == [nsg for _, _, nsg in tiles[:-1]]

    nc.compile()
    return nc


def _host_consts(inputs):
    bf = ml_dtypes.bfloat16
    M = _gcn_matrix(np.asarray(inputs["edge_index"]), np.asarray(inputs["edge_weight"]))
    adj = np.asarray(inputs["adj"], np.float32)
    bdM, m2e = _mix_consts(M, adj, G)
    bdM4, m2e4 = _mix_consts(M, adj, GT)
    W1 = np.asarray(inputs["W1"], np.float32)
    W2 = np.asarray(inputs["W2"], np.float32)
    W4 = np.asarray(inputs["W4"], np.float32)
    b1 = np.asarray(inputs["b1"], np.float32)
    b2 = np.asarray(inputs["b2"], np.float32)
    b4 = np.asarray(inputs["b4"], np.float32)
    b1b4 = np.stack([np.tile(b1, SGS), np.tile(b4, SGS)])
    cb = np.zeros((CIN, CB_COLS), np.float32)
    cb[:, 0:128] = W1
    cb[:, 128:192] = W2.T
    cb[0:MID, 192:320] = W4.T
    cb[0:R, 320:439] = bdM
    cb[0:R + 2, 439:558] = m2e
    cb[0:RT, 558:626] = bdM4
    cb[0:RT + 2, 626:694] = m2e4
    cb[0:2, 694:1206] = b1b4
    return {
        "cb": cb.astype(bf),
        "b2ab2": np.ascontiguousarray(
            np.stack([b2, SLOPE * b2], axis=1)),
    }


def _shard_x(vector: np.ndarray) -> np.ndarray:
    """Full [B, J, CIN] fp32 -> [N_CORES, CIN, ROWS] bf16 channel-major."""
    bf = ml_dtypes.bfloat16
    v = np.asarray(vector, np.float32).reshape(N_CORES, ROWS, CIN)
    return v.transpose(0, 2, 1).astype(bf)


def _assemble_out(outs) -> np.ndarray:
    """list of [CIN, ROWS] bf16 -> [B, J, CIN] fp32."""
    stacked = np.stack(outs)  # [N_CORES, CIN, ROWS]
    return (
        stacked.transpose(0, 2, 1)
        .astype(np.float32)
        .reshape(B, J, CIN)
    )


def kernel(**inputs) -> np.ndarray:
    from concourse.bass_utils import run_bass_kernel_spmd

    if "nc" not in _CACHE:
        _CACHE["nc"] = _build_bass()
    nc = _CACHE["nc"]

    consts = _host_consts(inputs)
    xs = _shard_x(inputs["vector"])
    in_maps = []
    for c in range(N_CORES):
        m = dict(consts)
        m["x"] = xs[c]
        in_maps.append(m)

    res = run_bass_kernel_spmd(nc, in_maps, core_ids=list(range(N_CORES)))
    return _assemble_out([res.results[c]["out"] for c in range(N_CORES)])
